# revision 50
# baseline (speedup 1.0000x reference)
"""Trainium2 Bass kernel for GQA self-attention (non-causal, RoPE).

Reference computation (B=2, T=2048, C=2048, 16 q-heads, 4 kv-heads, d=128):
    q = x @ Wq.T ; k = x @ Wk.T ; v = x @ Wv.T
    q, k <- RoPE(q, k)
    att = softmax(q k^T / sqrt(d))        (no causal mask)
    out = att @ v ; y = out @ Wo.T

Sharding: 8 cores = DP(batch)=2 x TP(kv-head group)=4.
Core c handles batch b=c//4, kv-group g=c%4 (q heads 4g..4g+3, kv head g).
Each core computes y_partial = out_g @ Wo[:, 512g:512(g+1)].T  [T, C];
host sums the 4 partials per batch element.

On-chip layout is "transposed": activations live as [feature, token] so the
token dim streams through the PE free dim at N=512 (fp32r => 1 cycle/row).
    qT = Wq_g @ x^T       [512, T]   (4 head-tiles of [128, T])
    kT = Wk_g @ x^T       [128, T]
    vT = Wv_g @ x^T -> PE-transpose -> v [T, 128] tiles
    S^T tiles [t,s] = kT_tile^T-free matmul: lhsT=kT[:,tblk], rhs=qT[:,h,schunk]
    E = exp(scale*S^T) via ACT (PSUM->SBUF)
    outT[d,s] += v[tblk]^T-style matmul: lhsT=v_tile, rhs=E_tile (PSUM accum)
    rowsum   += ones-matmul: lhsT=ones[128,128], rhs=E_tile
    outT_norm = outT * recip(rowsum)
    y[t,m] = lhsT=outT[:,h,tblk], rhs=WoT_g  (PSUM accum over 4 head blocks)
RoPE on-chip: the half-rotation partition swap is a PE matmul against a
host-supplied permutation matrix; the rotate-half sign is folded into the
host-precomputed sin table.
"""

import numpy as np

B = 2
T = 2048
C = 2048
HD = 128
N_HEAD = 16
N_KV = 4
KV_REP = N_HEAD // N_KV
ROPE_THETA = 10000.0
NCORES = 8
TP = 4  # kv-head groups
SCALE = 1.0 / float(np.sqrt(HD))

TCH = 512  # token chunk (matmul free dim)
NT = T // 128  # 16 token tiles of 128
NCH = T // TCH  # 4 token chunks
NKC = C // 128  # 16 contraction tiles

_CACHE = {}


def _build_nc():
    import concourse.bass as bass
    import concourse.mybir as mybir
    import concourse.tile as tile
    from concourse import bacc
    from concourse.masks import make_identity

    f32 = mybir.dt.float32
    f32r = mybir.dt.float32r

    def r(ap):
        return ap.bitcast(f32r)

    nc = bacc.Bacc(None)

    xT = nc.declare_dram_parameter("xT", [C, T], f32, isOutput=False)
    wqT = nc.declare_dram_parameter("wqT", [C, 4 * HD], f32, isOutput=False)
    wkT = nc.declare_dram_parameter("wkT", [C, HD], f32, isOutput=False)
    wvT = nc.declare_dram_parameter("wvT", [C, HD], f32, isOutput=False)
    woT = nc.declare_dram_parameter("woT", [4 * HD, C], f32, isOutput=False)
    cosT = nc.declare_dram_parameter("cosT", [HD, T], f32, isOutput=False)
    sinT = nc.declare_dram_parameter("sinT", [HD, T], f32, isOutput=False)
    onesd = nc.declare_dram_parameter("ones", [128, 128], f32, isOutput=False)
    permd = nc.declare_dram_parameter("perm", [128, 128], f32, isOutput=False)
    y = nc.declare_dram_parameter("y", [T, C], f32, isOutput=True)

    with tile.TileContext(nc) as tc:
        with (
            tc.tile_pool(name="persist", bufs=1) as persist,
            tc.tile_pool(name="small", bufs=1) as small,
        ):
            # Persistent SBUF tensors
            qT_sb = persist.tile([128, 4, T], f32r)  # [d, qhead, t]
            kT_sb = persist.tile([128, T], f32r)  # [d, t]
            v_sb = persist.tile([128, NT, HD], f32r)  # [t%128, tblk, d]
            outT_sb = persist.tile([128, 4, T], f32r)  # [d, qhead, s]
            ones_sb = small.tile([128, 128], f32r)
            id_sb = small.tile([128, 128], f32)
            perm_sb = small.tile([128, 128], f32r)

            # ---------------- Phase A: projections + RoPE ----------------
            with (
                tc.tile_pool(name="wA", bufs=1) as wA,
                tc.tile_pool(name="xload", bufs=4) as xload,
                tc.tile_pool(name="cossin", bufs=1) as cossin,
                tc.tile_pool(name="ropet", bufs=2) as ropet,
                tc.tile_pool(name="ppA", bufs=1, space="PSUM") as ppA,
                tc.tile_pool(name="ptvA", bufs=2, space="PSUM") as ptvA,
            ):
                # cos/sin live in their own late-released pool at the top of
                # the stack: phase B's first et/wo allocations then reuse the
                # early-released wA/xload ranges instead of waiting for the
                # last chunk's RoPE chain to finish reading cos/sin.
                cos_sb = cossin.tile([128, T], f32)
                sin_sb = cossin.tile([128, T], f32)
                wq_sb = wA.tile([128, NKC, 4 * HD], f32r)
                wk_sb = wA.tile([128, NKC, HD], f32r)
                wv_sb = wA.tile([128, NKC, HD], f32r)
                warm = wA.tile([128, 1], f32)
                # weight loads in 4-k-slice groups, spread across three
                # sequencers so DGE setup overlaps. Group 0 loads up front;
                # later groups are emitted just-in-time inside chunk 0's
                # k-loop so the first matmuls start as early as possible.
                def emit_wgroup(g4):
                    rsl = slice(g4 * 512, (g4 + 1) * 512)
                    nc.sync.dma_start(
                        out=wq_sb[:, 4 * g4 : 4 * g4 + 4, :],
                        in_=wqT[rsl, :]
                        .rearrange("(k p) d -> p k d", p=128)
                        .bitcast(f32r),
                    )
                    nc.scalar.dma_start(
                        out=wk_sb[:, 4 * g4 : 4 * g4 + 4, :],
                        in_=wkT[rsl, :]
                        .rearrange("(k p) d -> p k d", p=128)
                        .bitcast(f32r),
                    )
                    nc.gpsimd.dma_start(
                        out=wv_sb[:, 4 * g4 : 4 * g4 + 4, :],
                        in_=wvT[rsl, :]
                        .rearrange("(k p) d -> p k d", p=128)
                        .bitcast(f32r),
                    )

                emit_wgroup(0)
                nc.scalar.dma_start(out=ones_sb[:], in_=onesd[:].bitcast(f32r))
                nc.scalar.dma_start(out=perm_sb[:], in_=permd[:].bitcast(f32r))
                make_identity(nc, id_sb[:])
                # warm the ACT exp table set during the initial DMA wait so
                # the ~2.7us table load isn't paid at the first real softmax.
                nc.vector.memset(warm[:], 0.0)
                nc.scalar.activation(
                    out=warm[:], in_=warm[:],
                    func=mybir.ActivationFunctionType.Exp,
                )

                pending_tr = []  # delayed v-transposes (one chunk behind)
                for n in range(NCH):
                    tsl = bass.ts(n, TCH)
                    pq = [
                        ppA.tile([128, TCH], f32, tag=f"pq{j}", name=f"pq{j}")
                        for j in range(4)
                    ]
                    pk = ppA.tile([128, TCH], f32, tag="pk")
                    pv = ppA.tile([128, TCH], f32, tag="pv")
                    for kp in range(NKC // 2):
                        if n == 0 and kp in (0, 2, 4):
                            emit_wgroup(kp // 2 + 1)
                        # paired x^T loads: two 128-row k-slices per DMA
                        xt = xload.tile([128, 2, TCH], f32r, tag="xt")
                        nc.sync.dma_start(
                            out=xt[:],
                            in_=xT[kp * 256 : (kp + 1) * 256, tsl]
                            .rearrange("(kk p) t -> p kk t", p=128)
                            .bitcast(f32r),
                        )
                        for u in range(2):
                            k = 2 * kp + u
                            flags = dict(start=(k == 0), stop=(k == NKC - 1))
                            for j in range(4):
                                nc.tensor.matmul(
                                    pq[j][:],
                                    wq_sb[:, k, bass.ts(j, 128)],
                                    xt[:, u, :],
                                    **flags,
                                )
                            nc.tensor.matmul(
                                pk[:], wk_sb[:, k, :], xt[:, u, :], **flags
                            )
                            nc.tensor.matmul(
                                pv[:], wv_sb[:, k, :], xt[:, u, :], **flags
                            )

                    if n == 0:
                        # cos/sin are first needed by chunk 0's RoPE; loading
                        # here keeps them off the critical startup DMA path.
                        nc.scalar.dma_start(out=cos_sb[:], in_=cosT[:])
                        nc.gpsimd.dma_start(out=sin_sb[:], in_=sinT[:])

                    # emit previous chunk's v-transposes now: their vtmp input
                    # is long since ready, so they don't stall the PE queue.
                    for fn in pending_tr:
                        fn()
                    pending_tr = []

                    # RoPE: dst = psum*cos + perm(psum)*sin2, with the half
                    # rotation done as a PE permutation matmul (no DMA).
                    rope_jobs = [(pq[j], qT_sb[:, j, tsl]) for j in range(4)]
                    rope_jobs.append((pk, kT_sb[:, tsl]))
                    for psrc, dst in rope_jobs:
                        qa = ropet.tile([128, TCH], f32r, tag="qa")
                        nc.scalar.copy(qa[:], psrc[:])
                        qb = ptvA.tile([128, TCH], f32, tag="pp")
                        nc.tensor.matmul(qb[:], perm_sb[:], qa[:])
                        t1 = ropet.tile([128, TCH], f32, tag="t1")
                        nc.vector.tensor_mul(t1[:], qa[:].bitcast(f32), cos_sb[:, tsl])
                        t2 = ropet.tile([128, TCH], f32, tag="t2")
                        nc.vector.tensor_mul(t2[:], qb[:], sin_sb[:, tsl])
                        nc.vector.tensor_add(dst, t1[:], t2[:])

                    # v: transpose [d, t] -> [t, d] tiles via PE (delayed one
                    # chunk so the PE never waits on the vtmp ACT copy)
                    vtmp = ropet.tile([128, TCH], f32, tag="vtmp")
                    nc.scalar.copy(vtmp[:], pv[:])

                    def make_tr(vtmp=vtmp, n=n):
                        def emit():
                            for tl in range(TCH // 128):
                                ptv = ptvA.tile([128, 128], f32, tag="pp")
                                nc.tensor.transpose(
                                    ptv[:], vtmp[:, bass.ts(tl, 128)], id_sb[:]
                                )
                                nc.vector.tensor_copy(
                                    v_sb[:, n * (TCH // 128) + tl, :], ptv[:]
                                )
                        return emit

                    pending_tr.append(make_tr())
                for fn in pending_tr:
                    fn()

            # ---------------- Phase B: attention ----------------
            with tc.tile_pool(name="wC", bufs=1) as wC:
                # prefetch Wo during attention
                wo_sb = wC.tile([128, 4, C], f32r)
                nc.sync.dma_start(
                    out=wo_sb[:], in_=woT[:].rearrange("(k p) m -> p k m", p=128).bitcast(f32r)
                )

                with (
                    tc.tile_pool(name="epool", bufs=2) as epool,
                    tc.tile_pool(name="rpool", bufs=2) as rpool,
                    tc.tile_pool(name="pst", bufs=2, space="PSUM") as pstp,
                    tc.tile_pool(name="pacc", bufs=2, space="PSUM") as paccp,
                ):
                    for h in range(4):
                        for sc in range(NCH):
                            ssl = bass.ts(sc, TCH)
                            et = epool.tile([128, NT, TCH], f32r, tag="et")
                            pv_acc = paccp.tile([128, TCH], f32, tag="pvacc")
                            prs = paccp.tile([128, TCH], f32, tag="prs")

                            def qk_pair(tp):
                                # S^T for token tiles 2tp, 2tp+1 + exp -> et
                                pst = pstp.tile([128, 2, TCH], f32, tag="st")
                                for u in range(2):
                                    tt = 2 * tp + u
                                    nc.tensor.matmul(
                                        pst[:, u, :],
                                        kT_sb[:, bass.ts(tt, 128)],
                                        qT_sb[:, h, ssl],
                                    )
                                nc.scalar.activation(
                                    out=et[:, 2 * tp : 2 * tp + 2, :],
                                    in_=pst[:],
                                    func=mybir.ActivationFunctionType.Exp,
                                    scale=SCALE,
                                )

                            # software pipeline: QK/exp of pair tp+1 issues on
                            # PE before PV/ones of pair tp, so PE never sits
                            # behind the ACT exp it is waiting on.
                            qk_pair(0)
                            for tp in range(NT // 2):
                                if tp + 1 < NT // 2:
                                    qk_pair(tp + 1)
                                for u in range(2):
                                    tt = 2 * tp + u
                                    flags = dict(start=(tt == 0), stop=(tt == NT - 1))
                                    nc.tensor.matmul(
                                        pv_acc[:],
                                        v_sb[:, tt, :],
                                        et[:, tt, :],
                                        **flags,
                                    )
                                    nc.tensor.matmul(
                                        prs[:], ones_sb[:], et[:, tt, :], **flags
                                    )
                            rec = rpool.tile([128, TCH], f32, tag="rec")
                            nc.vector.reciprocal(rec[:], prs[:])
                            nc.vector.tensor_mul(outT_sb[:, h, ssl], pv_acc[:], rec[:])

                # ---------------- Phase C: output projection ----------------
                with (
                    tc.tile_pool(name="ypool", bufs=4) as ypool,
                    tc.tile_pool(name="pyp", bufs=4, space="PSUM") as pyp,
                ):
                    for i in range(NT):
                        last = i == NT - 1
                        ysb = ypool.tile([128, C], f32, tag="ysb")
                        for mc in range(NCH):
                            py = pyp.tile([128, TCH], f32, tag="py")
                            for kk in range(4):
                                nc.tensor.matmul(
                                    py[:],
                                    outT_sb[:, kk, bass.ts(i, 128)],
                                    wo_sb[:, kk, bass.ts(mc, TCH)],
                                    start=(kk == 0),
                                    stop=(kk == 3),
                                )
                            nc.scalar.copy(ysb[:, bass.ts(mc, TCH)], py[:])
                            if last:
                                # tail: per-chunk DMAs so the kernel end only
                                # waits on the final 512-column transfer
                                nc.sync.dma_start(
                                    out=y[i * 128 :, bass.ts(mc, TCH)],
                                    in_=ysb[:, bass.ts(mc, TCH)],
                                )
                        if not last:
                            nc.sync.dma_start(
                                out=y[i * 128 : (i + 1) * 128, :], in_=ysb[:]
                            )

    nc.compile()
    return nc


def _perm_matrix():
    # half-rotation permutation: out[d] = in[(d+64) % 128]; symmetric, so it
    # serves directly as the matmul lhsT.
    p = np.zeros((128, 128), dtype=np.float32)
    for i in range(128):
        p[i, (i + 64) % 128] = 1.0
    return p


def _rope_tables(start_pos):
    inv = (
        1.0
        / (ROPE_THETA ** (np.arange(0, HD, 2, dtype=np.float32) / np.float32(HD)))
    ).astype(np.float32)
    pos = np.arange(T, dtype=np.float32) + np.float32(start_pos)
    ang = pos[:, None] * inv[None, :]  # [T, 64]
    c = np.cos(ang, dtype=np.float32)
    s = np.sin(ang, dtype=np.float32)
    cosT = np.ascontiguousarray(np.concatenate([c, c], axis=1).T)  # [128, T]
    sin2 = np.concatenate([-s, s], axis=1)  # sign of rotate_half folded in
    sinT = np.ascontiguousarray(sin2.T)  # [128, T]
    return cosT, sinT


def kernel(x, Wq, Wk, Wv, Wo, start_pos):
    from concourse.bass_utils import run_bass_kernel_spmd

    if "nc" not in _CACHE:
        _CACHE["nc"] = _build_nc()
    nc = _CACHE["nc"]

    x = np.asarray(x, dtype=np.float32)
    Wq = np.asarray(Wq, dtype=np.float32)
    Wk = np.asarray(Wk, dtype=np.float32)
    Wv = np.asarray(Wv, dtype=np.float32)
    Wo = np.asarray(Wo, dtype=np.float32)
    cosT, sinT = _rope_tables(int(start_pos))
    xTs = [np.ascontiguousarray(x[b].T) for b in range(B)]
    ones = np.ones((128, 128), dtype=np.float32)
    perm = _perm_matrix()

    in_maps = []
    for c in range(NCORES):
        b, g = divmod(c, TP)
        in_maps.append(
            {
                "xT": xTs[b],
                "wqT": np.ascontiguousarray(Wq[512 * g : 512 * (g + 1), :].T),
                "wkT": np.ascontiguousarray(Wk[128 * g : 128 * (g + 1), :].T),
                "wvT": np.ascontiguousarray(Wv[128 * g : 128 * (g + 1), :].T),
                "woT": np.ascontiguousarray(Wo[:, 512 * g : 512 * (g + 1)].T),
                "cosT": cosT,
                "sinT": sinT,
                "ones": ones,
                "perm": perm,
            }
        )

    _CACHE["in_maps"] = in_maps
    res = run_bass_kernel_spmd(nc, in_maps, list(range(NCORES)))
    out = np.zeros((B, T, C), dtype=np.float32)
    for c in range(NCORES):
        out[c // TP] += res.results[c]["y"]
    return out


# revision 54
# speedup vs baseline: 1.0753x; 1.0753x over previous
"""Trainium2 Bass kernel for GQA self-attention (non-causal, RoPE).

Reference computation (B=2, T=2048, C=2048, 16 q-heads, 4 kv-heads, d=128):
    q = x @ Wq.T ; k = x @ Wk.T ; v = x @ Wv.T
    q, k <- RoPE(q, k)
    att = softmax(q k^T / sqrt(d))        (no causal mask)
    out = att @ v ; y = out @ Wo.T

Sharding: 8 cores = DP(batch)=2 x TP(kv-head group)=4.
Core c handles batch b=c//4, kv-group g=c%4 (q heads 4g..4g+3, kv head g).
Each core computes y_partial = out_g @ Wo[:, 512g:512(g+1)].T  [T, C];
host sums the 4 partials per batch element.

On-chip layout is "transposed": activations live as [feature, token] so the
token dim streams through the PE free dim at N=512 (fp32r => 1 cycle/row).
    qT = Wq_g @ x^T       [512, T]   (4 head-tiles of [128, T])
    kT = Wk_g @ x^T       [128, T]
    vT = Wv_g @ x^T -> PE-transpose -> v [T, 128] tiles
    S^T tiles [t,s] = kT_tile^T-free matmul: lhsT=kT[:,tblk], rhs=qT[:,h,schunk]
    E = exp(scale*S^T) via ACT (PSUM->SBUF)
    outT[d,s] += v[tblk]^T-style matmul: lhsT=v_tile, rhs=E_tile (PSUM accum)
    rowsum   += ones-matmul: lhsT=ones[128,128], rhs=E_tile
    outT_norm = outT * recip(rowsum)
    y[t,m] = lhsT=outT[:,h,tblk], rhs=WoT_g  (PSUM accum over 4 head blocks)
RoPE on-chip: the half-rotation partition swap is a PE matmul against a
host-supplied permutation matrix; the rotate-half sign is folded into the
host-precomputed sin table.
"""

import numpy as np

B = 2
T = 2048
C = 2048
HD = 128
N_HEAD = 16
N_KV = 4
KV_REP = N_HEAD // N_KV
ROPE_THETA = 10000.0
NCORES = 8
TP = 4  # kv-head groups
SCALE = 1.0 / float(np.sqrt(HD))

TCH = 512  # token chunk (matmul free dim)
NT = T // 128  # 16 token tiles of 128
NCH = T // TCH  # 4 token chunks
NKC = C // 128  # 16 contraction tiles

_CACHE = {}


def _build_nc():
    import concourse.bass as bass
    import concourse.mybir as mybir
    import concourse.tile as tile
    from concourse import bacc
    from concourse.masks import make_identity

    f32 = mybir.dt.float32
    f32r = mybir.dt.float32r

    def r(ap):
        return ap.bitcast(f32r)

    nc = bacc.Bacc(None)

    xT = nc.declare_dram_parameter("xT", [C, T], f32, isOutput=False)
    wqT = nc.declare_dram_parameter("wqT", [C, 4 * HD], f32, isOutput=False)
    wkT = nc.declare_dram_parameter("wkT", [C, HD], f32, isOutput=False)
    wvT = nc.declare_dram_parameter("wvT", [C, HD], f32, isOutput=False)
    woT = nc.declare_dram_parameter("woT", [4 * HD, C], f32, isOutput=False)
    cosT = nc.declare_dram_parameter("cosT", [HD, T], f32, isOutput=False)
    sinT = nc.declare_dram_parameter("sinT", [HD, T], f32, isOutput=False)
    onesd = nc.declare_dram_parameter("ones", [128, 128], f32, isOutput=False)
    permd = nc.declare_dram_parameter("perm", [128, 128], f32, isOutput=False)
    y = nc.declare_dram_parameter("y", [T, C], f32, isOutput=True)

    with tile.TileContext(nc) as tc:
        with (
            tc.tile_pool(name="persist", bufs=1) as persist,
            tc.tile_pool(name="small", bufs=1) as small,
        ):
            # Persistent SBUF tensors
            qT_sb = persist.tile([128, 4, T], f32r)  # [d, qhead, t]
            kT_sb = persist.tile([128, T], f32r)  # [d, t]
            v_sb = persist.tile([128, NT, HD], f32r)  # [t%128, tblk, d]
            outT_sb = persist.tile([128, 4, T], f32r)  # [d, qhead, s]
            ones_sb = small.tile([128, 128], f32r)
            id_sb = small.tile([128, 128], f32)
            perm_sb = small.tile([128, 128], f32r)

            # ---------------- Phase A: projections + RoPE ----------------
            with (
                tc.tile_pool(name="wA", bufs=1) as wA,
                tc.tile_pool(name="xload", bufs=4) as xload,
                tc.tile_pool(name="cossin", bufs=1) as cossin,
                tc.tile_pool(name="ropet", bufs=2) as ropet,
                tc.tile_pool(name="ppA", bufs=1, space="PSUM") as ppA,
                tc.tile_pool(name="ptvA", bufs=2, space="PSUM") as ptvA,
            ):
                # cos/sin live in their own late-released pool at the top of
                # the stack: phase B's first et/wo allocations then reuse the
                # early-released wA/xload ranges instead of waiting for the
                # last chunk's RoPE chain to finish reading cos/sin.
                cos_sb = cossin.tile([128, T], f32)
                sin_sb = cossin.tile([128, T], f32)
                wq_sb = wA.tile([128, NKC, 4 * HD], f32r)
                wk_sb = wA.tile([128, NKC, HD], f32r)
                wv_sb = wA.tile([128, NKC, HD], f32r)
                warm = wA.tile([128, 1], f32)
                # weight loads in 4-k-slice groups, spread across three
                # sequencers so DGE setup overlaps. Group 0 loads up front;
                # later groups are emitted just-in-time inside chunk 0's
                # k-loop so the first matmuls start as early as possible.
                def emit_wgroup(g4):
                    rsl = slice(g4 * 512, (g4 + 1) * 512)
                    nc.sync.dma_start(
                        out=wq_sb[:, 4 * g4 : 4 * g4 + 4, :],
                        in_=wqT[rsl, :]
                        .rearrange("(k p) d -> p k d", p=128)
                        .bitcast(f32r),
                    )
                    nc.scalar.dma_start(
                        out=wk_sb[:, 4 * g4 : 4 * g4 + 4, :],
                        in_=wkT[rsl, :]
                        .rearrange("(k p) d -> p k d", p=128)
                        .bitcast(f32r),
                    )
                    nc.gpsimd.dma_start(
                        out=wv_sb[:, 4 * g4 : 4 * g4 + 4, :],
                        in_=wvT[rsl, :]
                        .rearrange("(k p) d -> p k d", p=128)
                        .bitcast(f32r),
                    )

                emit_wgroup(0)
                nc.scalar.dma_start(out=ones_sb[:], in_=onesd[:].bitcast(f32r))
                nc.scalar.dma_start(out=perm_sb[:], in_=permd[:].bitcast(f32r))
                make_identity(nc, id_sb[:])
                # warm the ACT exp table set during the initial DMA wait so
                # the ~2.7us table load isn't paid at the first real softmax.
                nc.vector.memset(warm[:], 0.0)
                nc.scalar.activation(
                    out=warm[:], in_=warm[:],
                    func=mybir.ActivationFunctionType.Exp,
                )

                pending_tr = []  # delayed v-transposes (one chunk behind)
                for n in range(NCH):
                    tsl = bass.ts(n, TCH)
                    pq = [
                        ppA.tile([128, TCH], f32, tag=f"pq{j}", name=f"pq{j}")
                        for j in range(4)
                    ]
                    pk = ppA.tile([128, TCH], f32, tag="pk")
                    pv = ppA.tile([128, TCH], f32, tag="pv")
                    for kp in range(NKC // 2):
                        if n == 0 and kp in (0, 2, 4):
                            emit_wgroup(kp // 2 + 1)
                        # paired x^T loads: two 128-row k-slices per DMA
                        xt = xload.tile([128, 2, TCH], f32r, tag="xt")
                        nc.sync.dma_start(
                            out=xt[:],
                            in_=xT[kp * 256 : (kp + 1) * 256, tsl]
                            .rearrange("(kk p) t -> p kk t", p=128)
                            .bitcast(f32r),
                        )
                        for u in range(2):
                            k = 2 * kp + u
                            flags = dict(start=(k == 0), stop=(k == NKC - 1))
                            for j in range(4):
                                nc.tensor.matmul(
                                    pq[j][:],
                                    wq_sb[:, k, bass.ts(j, 128)],
                                    xt[:, u, :],
                                    **flags,
                                )
                            nc.tensor.matmul(
                                pk[:], wk_sb[:, k, :], xt[:, u, :], **flags
                            )
                            nc.tensor.matmul(
                                pv[:], wv_sb[:, k, :], xt[:, u, :], **flags
                            )

                    if n == 0:
                        # cos/sin are first needed by chunk 0's RoPE; loading
                        # here keeps them off the critical startup DMA path.
                        nc.scalar.dma_start(out=cos_sb[:], in_=cosT[:])
                        nc.gpsimd.dma_start(out=sin_sb[:], in_=sinT[:])

                    # emit previous chunk's v-transposes now: their vtmp input
                    # is long since ready, so they don't stall the PE queue.
                    for fn in pending_tr:
                        fn()
                    pending_tr = []

                    # RoPE: dst = psum*cos + perm(psum)*sin2, with the half
                    # rotation done as a PE permutation matmul (no DMA).
                    rope_jobs = [(pq[j], qT_sb[:, j, tsl]) for j in range(4)]
                    rope_jobs.append((pk, kT_sb[:, tsl]))
                    for psrc, dst in rope_jobs:
                        qa = ropet.tile([128, TCH], f32r, tag="qa")
                        nc.scalar.copy(qa[:], psrc[:])
                        qb = ptvA.tile([128, TCH], f32, tag="pp")
                        nc.tensor.matmul(qb[:], perm_sb[:], qa[:])
                        t1 = ropet.tile([128, TCH], f32, tag="t1")
                        nc.vector.tensor_mul(t1[:], qa[:].bitcast(f32), cos_sb[:, tsl])
                        t2 = ropet.tile([128, TCH], f32, tag="t2")
                        nc.vector.tensor_mul(t2[:], qb[:], sin_sb[:, tsl])
                        nc.vector.tensor_add(dst, t1[:], t2[:])

                    # v: transpose [d, t] -> [t, d] tiles via PE (delayed one
                    # chunk so the PE never waits on the vtmp ACT copy)
                    vtmp = ropet.tile([128, TCH], f32, tag="vtmp")
                    nc.scalar.copy(vtmp[:], pv[:])

                    def make_tr(vtmp=vtmp, n=n):
                        def emit():
                            for tl in range(TCH // 128):
                                ptv = ptvA.tile([128, 128], f32, tag="pp")
                                nc.tensor.transpose(
                                    ptv[:], vtmp[:, bass.ts(tl, 128)], id_sb[:]
                                )
                                nc.vector.tensor_copy(
                                    v_sb[:, n * (TCH // 128) + tl, :], ptv[:]
                                )
                        return emit

                    pending_tr.append(make_tr())
                for fn in pending_tr:
                    fn()

            # ---------------- Phase B: attention ----------------
            with tc.tile_pool(name="wC", bufs=1) as wC:
                # prefetch Wo during attention
                wo_sb = wC.tile([128, 4, C], f32r)
                nc.sync.dma_start(
                    out=wo_sb[:], in_=woT[:].rearrange("(k p) m -> p k m", p=128).bitcast(f32r)
                )

                with (
                    tc.tile_pool(name="epool", bufs=2) as epool,
                    tc.tile_pool(name="rpool", bufs=2) as rpool,
                    tc.tile_pool(name="pst", bufs=2, space="PSUM") as pstp,
                    tc.tile_pool(name="pacc", bufs=2, space="PSUM") as paccp,
                ):
                    for h in range(4):
                        for sc in range(NCH):
                            ssl = bass.ts(sc, TCH)
                            et = epool.tile([128, NT, TCH], f32r, tag="et")
                            pv_acc = paccp.tile([128, TCH], f32, tag="pvacc")
                            prs = paccp.tile([128, TCH], f32, tag="prs")

                            def qk_pair(tp):
                                # S^T for token tiles 2tp, 2tp+1 + exp -> et
                                pst = pstp.tile([128, 2, TCH], f32, tag="st")
                                for u in range(2):
                                    tt = 2 * tp + u
                                    nc.tensor.matmul(
                                        pst[:, u, :],
                                        kT_sb[:, bass.ts(tt, 128)],
                                        qT_sb[:, h, ssl],
                                    )
                                nc.scalar.activation(
                                    out=et[:, 2 * tp : 2 * tp + 2, :],
                                    in_=pst[:],
                                    func=mybir.ActivationFunctionType.Exp,
                                    scale=SCALE,
                                )

                            # software pipeline: QK/exp of pair tp+1 issues on
                            # PE before PV of pair tp, so PE never sits behind
                            # the ACT exp it is waiting on.
                            qk_pair(0)
                            for tp in range(NT // 2):
                                if tp + 1 < NT // 2:
                                    qk_pair(tp + 1)
                                for u in range(2):
                                    tt = 2 * tp + u
                                    nc.tensor.matmul(
                                        pv_acc[:],
                                        v_sb[:, tt, :],
                                        et[:, tt, :],
                                        start=(tt == 0),
                                        stop=(tt == NT - 1),
                                    )
                                # rowsum: pre-sum pairs (and pair-of-pairs) on
                                # the idle DVE, then one ones-matmul per QUAD
                                # -- quarters the PE cost of the softmax
                                # denominators.
                                nc.vector.tensor_add(
                                    et[:, 2 * tp, :],
                                    et[:, 2 * tp, :],
                                    et[:, 2 * tp + 1, :],
                                )
                                if tp % 2 == 1:
                                    q0 = 2 * (tp - 1)
                                    nc.vector.tensor_add(
                                        et[:, q0, :],
                                        et[:, q0, :],
                                        et[:, 2 * tp, :],
                                    )
                                    nc.tensor.matmul(
                                        prs[:],
                                        ones_sb[:],
                                        et[:, q0, :],
                                        start=(tp == 1),
                                        stop=(tp == NT // 2 - 1),
                                    )
                            rec = rpool.tile([128, TCH], f32, tag="rec")
                            nc.vector.reciprocal(rec[:], prs[:])
                            nc.vector.tensor_mul(outT_sb[:, h, ssl], pv_acc[:], rec[:])

                # ---------------- Phase C: output projection ----------------
                with (
                    tc.tile_pool(name="ypool", bufs=4) as ypool,
                    tc.tile_pool(name="pyp", bufs=4, space="PSUM") as pyp,
                ):
                    for i in range(NT):
                        last = i == NT - 1
                        ysb = ypool.tile([128, C], f32, tag="ysb")
                        for mc in range(NCH):
                            py = pyp.tile([128, TCH], f32, tag="py")
                            for kk in range(4):
                                nc.tensor.matmul(
                                    py[:],
                                    outT_sb[:, kk, bass.ts(i, 128)],
                                    wo_sb[:, kk, bass.ts(mc, TCH)],
                                    start=(kk == 0),
                                    stop=(kk == 3),
                                )
                            nc.scalar.copy(ysb[:, bass.ts(mc, TCH)], py[:])
                            if last:
                                # tail: per-chunk DMAs so the kernel end only
                                # waits on the final 512-column transfer
                                nc.sync.dma_start(
                                    out=y[i * 128 :, bass.ts(mc, TCH)],
                                    in_=ysb[:, bass.ts(mc, TCH)],
                                )
                        if not last:
                            nc.sync.dma_start(
                                out=y[i * 128 : (i + 1) * 128, :], in_=ysb[:]
                            )

    nc.compile()
    return nc


def _perm_matrix():
    # half-rotation permutation: out[d] = in[(d+64) % 128]; symmetric, so it
    # serves directly as the matmul lhsT.
    p = np.zeros((128, 128), dtype=np.float32)
    for i in range(128):
        p[i, (i + 64) % 128] = 1.0
    return p


def _rope_tables(start_pos):
    inv = (
        1.0
        / (ROPE_THETA ** (np.arange(0, HD, 2, dtype=np.float32) / np.float32(HD)))
    ).astype(np.float32)
    pos = np.arange(T, dtype=np.float32) + np.float32(start_pos)
    ang = pos[:, None] * inv[None, :]  # [T, 64]
    c = np.cos(ang, dtype=np.float32)
    s = np.sin(ang, dtype=np.float32)
    cosT = np.ascontiguousarray(np.concatenate([c, c], axis=1).T)  # [128, T]
    sin2 = np.concatenate([-s, s], axis=1)  # sign of rotate_half folded in
    sinT = np.ascontiguousarray(sin2.T)  # [128, T]
    return cosT, sinT


def kernel(x, Wq, Wk, Wv, Wo, start_pos):
    from concourse.bass_utils import run_bass_kernel_spmd

    if "nc" not in _CACHE:
        _CACHE["nc"] = _build_nc()
    nc = _CACHE["nc"]

    x = np.asarray(x, dtype=np.float32)
    Wq = np.asarray(Wq, dtype=np.float32)
    Wk = np.asarray(Wk, dtype=np.float32)
    Wv = np.asarray(Wv, dtype=np.float32)
    Wo = np.asarray(Wo, dtype=np.float32)
    cosT, sinT = _rope_tables(int(start_pos))
    xTs = [np.ascontiguousarray(x[b].T) for b in range(B)]
    ones = np.ones((128, 128), dtype=np.float32)
    perm = _perm_matrix()

    in_maps = []
    for c in range(NCORES):
        b, g = divmod(c, TP)
        in_maps.append(
            {
                "xT": xTs[b],
                "wqT": np.ascontiguousarray(Wq[512 * g : 512 * (g + 1), :].T),
                "wkT": np.ascontiguousarray(Wk[128 * g : 128 * (g + 1), :].T),
                "wvT": np.ascontiguousarray(Wv[128 * g : 128 * (g + 1), :].T),
                "woT": np.ascontiguousarray(Wo[:, 512 * g : 512 * (g + 1)].T),
                "cosT": cosT,
                "sinT": sinT,
                "ones": ones,
                "perm": perm,
            }
        )

    _CACHE["in_maps"] = in_maps
    res = run_bass_kernel_spmd(nc, in_maps, list(range(NCORES)))
    out = np.zeros((B, T, C), dtype=np.float32)
    for c in range(NCORES):
        out[c // TP] += res.results[c]["y"]
    return out
